# revision 3
# baseline (speedup 1.0000x reference)
"""LCAOInteraction kernel for 8 trn2 cores.

Strategy (edge/graph-parallel): edges are sharded contiguously across the 8
cores (25000 edges each). The dense coefficient transform
c2 = silu(silu(cji) @ W2.T) @ W3.T -- the dominant memory + FLOP term -- runs
on the NeuronCores in bf16 via a Bass/Tile kernel:

  - host applies silu(cji), casts to bf16 and pre-transposes each shard into a
    partition-packed [128, COLSH] layout (two (e,d)-rows per packed column:
    even rows on partitions 0:64, odd rows on 64:128),
  - block-diagonal weights turn the CF=64-contraction matmuls into
    full-128-partition matmuls (halves PE work vs the 64-wide layout),
  - per 1024-col pair: mm1 -> PSUM, silu -> bf16, mm2 -> PSUM, one DVE copy,
    2 MB DMA chunks; emission is software-pipelined (lag 1 pair) so PE never
    stalls behind the same pair's activation,
  - everything is bf16 in/out (fp32 accumulation in PSUM), which is safe: the
    final-output absmax rel-err of the bf16 path is ~2e-3 vs the 2e-2 gate.

The index-dependent graph plumbing (gathers / segment sums / small epilogue
matmuls) runs on the host around the device stage with in-place float32
numpy. Device failures fall back to a full numpy path so the kernel always
returns a correct full-shape output.
"""
import sys
import numpy as np

sys.path.insert(0, "/opt/trn_rl_repo")

import ml_dtypes

BF16 = ml_dtypes.bfloat16

N, E, T, NORB, H, CF, C = 10000, 200000, 400000, 9, 128, 64, 32
NCORES = 8
ES = E // NCORES               # 25000 edges per core
COLS = ES * NORB               # 225000 (e,d) rows per core
TW = 512                       # matmul tile width
PW = 2 * TW                    # pair width
CHUNK = 8192                   # DMA chunk (packed cols)
COLSH = 112640                 # packed cols per core (>= COLS/2, % PW == 0)
IN_SPLITS = 4

LAST_EXEC_NS = [0]
LAST_NC = [None]


def _silu(x):
    return x / (1.0 + np.exp(-x))


def _build_c2(ncores=NCORES):
    """Bass module: c2pk = pack(silu(in @ W2bd) @ W3bd) on one core."""
    import concourse.bacc as bacc
    import concourse.mybir as mybir
    import concourse.tile as tile

    F32 = mybir.dt.float32
    B16 = mybir.dt.bfloat16
    SILU = mybir.ActivationFunctionType.Silu

    chunks = []
    c0 = 0
    while c0 < COLSH:
        w = min(CHUNK, COLSH - c0)
        chunks.append((c0, w))
        c0 += w

    nc = bacc.Bacc("TRN2", target_bir_lowering=False, debug=False,
                   enable_asserts=False, num_devices=ncores)
    t_in = nc.dram_tensor("cji_rows", (128, COLSH), B16, kind="ExternalInput")
    t_w2 = nc.dram_tensor("w2bd", (128, CF), B16, kind="ExternalInput")
    t_w3 = nc.dram_tensor("w3bd", (CF, 128), B16, kind="ExternalInput")
    t_out = nc.dram_tensor("c2pk", (128, COLSH), B16, kind="ExternalOutput")

    with tile.TileContext(nc) as tc:
        with tc.tile_pool(name="w", bufs=1) as wp, \
             tc.tile_pool(name="io", bufs=3) as io, \
             tc.tile_pool(name="sb", bufs=4) as sb, \
             tc.tile_pool(name="ps", bufs=2, space="PSUM") as ps1, \
             tc.tile_pool(name="ps2", bufs=2, space="PSUM") as ps2:
            w2 = wp.tile([128, CF], B16)
            nc.sync.dma_start(out=w2[:], in_=t_w2[:, :])
            # w3 in both partition halves: matmul needs
            # lhsT.base_partition() == rhs.base_partition()
            w3 = wp.tile([128, 128], B16)
            nc.sync.dma_start(out=w3[0:CF, :], in_=t_w3[:, :])
            nc.sync.dma_start(out=w3[CF:128, :], in_=t_w3[:, :])

            pairs = []
            for ci, (c0, w) in enumerate(chunks):
                for ja in range(0, w, PW):
                    pairs.append((ci, ja))
            x_tiles, o_tiles = {}, {}
            done = {ci: 0 for ci in range(len(chunks))}
            npairs = {ci: w // PW for ci, (c0, w) in enumerate(chunks)}

            def start_chunk(ci):
                c0, w = chunks[ci]
                x = io.tile([128, CHUNK], B16, tag="x", name="x")
                sw = w // IN_SPLITS
                for s in range(IN_SPLITS):
                    lo = s * sw
                    nc.sync.dma_start(out=x[:, lo:lo + sw],
                                      in_=t_in[:, c0 + lo:c0 + lo + sw])
                x_tiles[ci] = x
                o_tiles[ci] = io.tile([128, CHUNK], B16, tag="o", name="o")

            def emit_mm1(k):
                ci, ja = pairs[k]
                if ci not in x_tiles:
                    start_chunk(ci)
                s1 = x_tiles[ci]
                p1 = ps1.tile([128, TW], F32, space="PSUM", tag="p1", name="p1")
                nc.tensor.matmul(out=p1[0:CF, :], lhsT=w2[:],
                                 rhs=s1[:, ja:ja + TW], start=True, stop=True)
                nc.tensor.matmul(out=p1[CF:128, :], lhsT=w2[:],
                                 rhs=s1[:, ja + TW:ja + PW], start=True,
                                 stop=True)
                return p1

            def emit_rest(k, p1):
                ci, ja = pairs[k]
                s2 = sb.tile([128, TW], B16, tag="s2", name="s2")
                nc.scalar.activation(out=s2[:, :], in_=p1[:, :], func=SILU)
                p2 = ps2.tile([128, PW], F32, space="PSUM", tag="p2", name="p2")
                nc.tensor.matmul(out=p2[:, 0:TW], lhsT=w3[0:CF, :],
                                 rhs=s2[0:CF, :], start=True, stop=True)
                nc.tensor.matmul(out=p2[:, TW:PW], lhsT=w3[CF:128, :],
                                 rhs=s2[CF:128, :], start=True, stop=True)
                o = o_tiles[ci]
                nc.vector.tensor_copy(out=o[:, ja:ja + PW], in_=p2[:, :])
                done[ci] += 1
                if done[ci] == npairs[ci]:
                    c0, w = chunks[ci]
                    nc.sync.dma_start(out=t_out[:, c0:c0 + w], in_=o[:, :w])

            prev = None
            for k in range(len(pairs)):
                p1 = emit_mm1(k)
                if prev is not None:
                    emit_rest(k - 1, prev)
                prev = p1
            emit_rest(len(pairs) - 1, prev)

    nc.compile()
    return nc


def _blockdiag2(a):
    r, c = a.shape
    out = np.zeros((2 * r, 2 * c), a.dtype)
    out[:r, :c] = a
    out[r:, c:] = a
    return out


def _c2_on_device(cji, W2, W3):
    """c2[e,d,:] = silu(silu(cji[e,d,:]) @ W2.T) @ W3.T on 8 NeuronCores."""
    from concourse.bass_utils import run_bass_kernel_spmd

    nc = _build_c2()
    LAST_NC[0] = nc

    w2bd = _blockdiag2(np.ascontiguousarray(W2.T)).astype(BF16)   # (128, 64)
    w3bd = _blockdiag2(np.ascontiguousarray(W3.T)).astype(BF16)   # (64, 128)

    # silu in fewer passes: s = x * sigmoid(x)
    flat = cji.reshape(E * NORB, CF)
    s1 = np.exp(-flat)
    s1 += 1.0
    np.reciprocal(s1, out=s1)
    s1 *= flat
    s1 = s1.astype(BF16)                                          # (1.8M, 64)
    in_maps = []
    for c in range(NCORES):
        shard = s1[c * COLS:(c + 1) * COLS]                       # (COLS, 64)
        xpk = np.empty((128, COLSH), BF16)
        xpk[:, :COLS // 2] = shard.reshape(COLS // 2, 128).T
        xpk[:, COLS // 2:] = 0
        in_maps.append({"cji_rows": xpk, "w2bd": w2bd, "w3bd": w3bd})

    res = run_bass_kernel_spmd(nc, in_maps, core_ids=list(range(NCORES)))
    if res.exec_time_ns:
        LAST_EXEC_NS[0] += int(res.exec_time_ns)

    c2 = np.empty((E * NORB, 2 * C), np.float32)
    for c in range(NCORES):
        pk = res.results[c]["c2pk"]                               # (128, COLSH)
        rows = c2[c * COLS:(c + 1) * COLS]
        rows[0::2] = pk[0:64, :COLS // 2].T
        rows[1::2] = pk[64:128, :COLS // 2].T
    return c2.reshape(E, NORB, 2 * C)


def kernel(x, cji, cutoff_w, rb, shb,
           W1, b1, W2, W3, W4, b4, W5, b5, W6, b6, W7,
           idx_i, idx_j, tri_idx_k, edge_idx_kj, edge_idx_ji):
    LAST_EXEC_NS[0] = 0
    x = np.asarray(x, np.float32)
    cji = np.asarray(cji, np.float32)
    rb = np.asarray(rb, np.float32)
    shb = np.asarray(shb, np.float32)
    cw = np.asarray(cutoff_w, np.float32)
    W1, W2, W3, W4, W5, W6, W7 = (np.asarray(w, np.float32)
                                  for w in (W1, W2, W3, W4, W5, W6, W7))
    b1, b4, b5, b6 = (np.asarray(b, np.float32) for b in (b1, b4, b5, b6))
    ii = np.asarray(idx_i).astype(np.int64)
    jj = np.asarray(idx_j).astype(np.int64)
    kk = np.asarray(tri_idx_k).astype(np.int64)
    ekj = np.asarray(edge_idx_kj).astype(np.int64)
    eji = np.asarray(edge_idx_ji).astype(np.int64)

    # dense coefficient transform: device (8-way edge shards), host fallback
    try:
        c2 = _c2_on_device(cji, W2, W3)
    except Exception as e:  # noqa: BLE001
        print(f"[kernel] device path failed ({type(e).__name__}: {e}); "
              f"falling back to host", file=sys.stderr)
        c2 = (_silu(_silu(cji) @ W2.T) @ W3.T).astype(np.float32)

    # ---- host graph pipeline (in-place fp32 numpy) ----
    h = x @ W1.T + b1
    xh = h[:, :C]
    sig_xk = 1.0 / (1.0 + np.exp(-h[:, C:]))                  # (N, C)
    rb_w = rb * cw[:, None]                                   # (E, 9)
    cji_c = c2[..., :C]
    ckj = c2[..., C:]
    # fold 1/||ckj[e,d]|| into the triplet coefficients
    rn = np.einsum('edh,edh->ed', ckj, ckj)
    np.sqrt(rn, out=rn)
    np.maximum(rn, 1e-12, out=rn)
    np.reciprocal(rn, out=rn)
    coef = rb_w[ekj]
    coef *= shb
    coef *= rn[ekj]
    g = ckj[ekj]                                              # (T, 9, C)
    tbo = (coef[:, None, :] @ g)[:, 0, :]                     # (T, C)
    del g
    nrm = np.einsum('th,th->t', tbo, tbo)
    np.sqrt(nrm, out=nrm)
    np.maximum(nrm, 1e-12, out=nrm)
    np.reciprocal(nrm, out=nrm)
    tw = tbo
    tw *= nrm[:, None]
    tw *= sig_xk[kk]
    agg = np.zeros((E, C), np.float32)
    np.add.at(agg, eji, tw)
    tbw = _silu(agg) @ W4.T + b4
    tbw += 1.0                                                # (E, C)
    # lcao = l2n(sum_d rb_w[e,d] * l2n(cji_c[e,d,:] * tbw[e,:]))
    n2 = ((cji_c * cji_c) @ (tbw * tbw)[:, :, None])[:, :, 0]  # (E, 9)
    np.sqrt(n2, out=n2)
    np.maximum(n2, 1e-12, out=n2)
    np.reciprocal(n2, out=n2)
    coef2 = rb_w * n2
    lc = (coef2[:, None, :] @ cji_c)[:, 0, :]                 # (E, C)
    lc *= tbw
    nrm2 = np.einsum('eh,eh->e', lc, lc)
    np.sqrt(nrm2, out=nrm2)
    np.maximum(nrm2, 1e-12, out=nrm2)
    np.reciprocal(nrm2, out=nrm2)
    lc *= nrm2[:, None]
    nf = np.empty((E, 2 * C), np.float32)
    nf[:, :C] = xh[ii]
    nf[:, C:] = xh[jj]
    nf = _silu(_silu(nf) @ W5.T + b5) @ W6.T + b6
    msg = lc
    msg *= nf
    node = np.zeros((N, C), np.float32)
    np.add.at(node, ii, msg)
    out = x + node @ W7.T
    return out.astype(np.float32)


# revision 9
# speedup vs baseline: 1.2661x; 1.2661x over previous
"""LCAOInteraction kernel for 8 trn2 cores.

Strategy (edge/graph-parallel): edges are sharded contiguously across the 8
cores (25000 edges each). The dense coefficient transform
c2 = silu(silu(cji) @ W2.T) @ W3.T -- the dominant memory + FLOP term -- runs
on the NeuronCores in bf16 via a Bass/Tile kernel:

  - host applies silu(cji), casts to bf16 and pre-transposes each shard into a
    partition-packed [128, COLSH] layout (two (e,d)-rows per packed column:
    even rows on partitions 0:64, odd rows on 64:128),
  - block-diagonal weights turn the CF=64-contraction matmuls into
    full-128-partition matmuls (halves PE work vs the 64-wide layout),
  - per 1024-col pair: mm1 -> PSUM, silu -> bf16, mm2 -> PSUM, one PSUM->SBUF
    copy (balanced 5:2 between DVE and ACT so neither engine binds), 2 MB DMA
    chunks split into 8 sub-DMAs; emission is software-pipelined (lag 1 pair)
    so PE never stalls behind the same pair's activation; output DMAs go out
    on the SWDGE (gpsimd) queue so they never block input loads on HWDGE,
  - input is bf16, output is fp8(e4m3) with fp32 accumulation in PSUM. The
    final-output absmax rel-err of this path is ~1.2e-2 vs the 2e-2 gate
    (bf16-out would be ~2e-3); fp8 output halves the store traffic, and the
    kernel is DMA-bound, so this is a ~20% device-time win.

The index-dependent graph plumbing (gathers / segment sums / small epilogue
matmuls) runs on the host around the device stage with in-place float32
numpy. Device failures fall back to a full numpy path so the kernel always
returns a correct full-shape output.
"""
import sys
import numpy as np

sys.path.insert(0, "/opt/trn_rl_repo")

import ml_dtypes

BF16 = ml_dtypes.bfloat16

N, E, T, NORB, H, CF, C = 10000, 200000, 400000, 9, 128, 64, 32
NCORES = 8
ES = E // NCORES               # 25000 edges per core
COLS = ES * NORB               # 225000 (e,d) rows per core
TW = 512                       # matmul tile width
PW = 2 * TW                    # pair width
CHUNK = 8192                   # DMA chunk (packed cols)
COLSH = 112640                 # packed cols per core (>= COLS/2, % PW == 0)
IN_SPLITS = 8                  # sub-DMAs per input chunk (finer deps)
OUT_SPLITS = 4                 # output flushes per chunk
ACT_COPY_MOD = (5, 7)          # pairs k with k%7>=5 copy PSUM->SBUF on ACT
FP8 = ml_dtypes.float8_e4m3

LAST_EXEC_NS = [0]
LAST_NC = [None]


def _silu(x):
    return x / (1.0 + np.exp(-x))


def _build_c2(ncores=NCORES):
    """Bass module: c2pk = pack(silu(in @ W2bd) @ W3bd) on one core."""
    import concourse.bacc as bacc
    import concourse.mybir as mybir
    import concourse.tile as tile

    F32 = mybir.dt.float32
    B16 = mybir.dt.bfloat16
    F8 = mybir.dt.float8e4
    SILU = mybir.ActivationFunctionType.Silu

    chunks = []
    c0 = 0
    while c0 < COLSH:
        w = min(CHUNK, COLSH - c0)
        chunks.append((c0, w))
        c0 += w

    nc = bacc.Bacc("TRN2", target_bir_lowering=False, debug=False,
                   enable_asserts=False, num_devices=ncores)
    t_in = nc.dram_tensor("cji_rows", (128, COLSH), B16, kind="ExternalInput")
    t_w2 = nc.dram_tensor("w2bd", (128, CF), B16, kind="ExternalInput")
    t_w3 = nc.dram_tensor("w3bd", (CF, 128), B16, kind="ExternalInput")
    t_out = nc.dram_tensor("c2pk", (128, COLSH), F8, kind="ExternalOutput")

    with tile.TileContext(nc) as tc:
        with tc.tile_pool(name="w", bufs=1) as wp, \
             tc.tile_pool(name="io", bufs=3) as io, \
             tc.tile_pool(name="sb", bufs=4) as sb, \
             tc.tile_pool(name="ps", bufs=2, space="PSUM") as ps1, \
             tc.tile_pool(name="ps2", bufs=2, space="PSUM") as ps2:
            w2 = wp.tile([128, CF], B16)
            nc.sync.dma_start(out=w2[:], in_=t_w2[:, :])
            # w3 in both partition halves: matmul needs
            # lhsT.base_partition() == rhs.base_partition()
            w3 = wp.tile([128, 128], B16)
            nc.sync.dma_start(out=w3[0:CF, :], in_=t_w3[:, :])
            nc.sync.dma_start(out=w3[CF:128, :], in_=t_w3[:, :])

            pairs = []
            for ci, (c0, w) in enumerate(chunks):
                for ja in range(0, w, PW):
                    pairs.append((ci, ja))
            x_tiles, o_tiles = {}, {}
            done = {ci: 0 for ci in range(len(chunks))}
            npairs = {ci: w // PW for ci, (c0, w) in enumerate(chunks)}

            def start_chunk(ci):
                c0, w = chunks[ci]
                x = io.tile([128, CHUNK], B16, tag="x", name="x")
                sw = w // IN_SPLITS
                for s in range(IN_SPLITS):
                    lo = s * sw
                    nc.sync.dma_start(out=x[:, lo:lo + sw],
                                      in_=t_in[:, c0 + lo:c0 + lo + sw])
                x_tiles[ci] = x
                o_tiles[ci] = io.tile([128, CHUNK], F8, tag="o", name="o")

            def emit_mm1(k):
                ci, ja = pairs[k]
                if ci not in x_tiles:
                    start_chunk(ci)
                s1 = x_tiles[ci]
                p1 = ps1.tile([128, TW], F32, space="PSUM", tag="p1", name="p1")
                nc.tensor.matmul(out=p1[0:CF, :], lhsT=w2[:],
                                 rhs=s1[:, ja:ja + TW], start=True, stop=True)
                nc.tensor.matmul(out=p1[CF:128, :], lhsT=w2[:],
                                 rhs=s1[:, ja + TW:ja + PW], start=True,
                                 stop=True)
                return p1

            def emit_rest(k, p1):
                ci, ja = pairs[k]
                s2 = sb.tile([128, TW], B16, tag="s2", name="s2")
                nc.scalar.activation(out=s2[:, :], in_=p1[:, :], func=SILU)
                p2 = ps2.tile([128, PW], F32, space="PSUM", tag="p2", name="p2")
                nc.tensor.matmul(out=p2[:, 0:TW], lhsT=w3[0:CF, :],
                                 rhs=s2[0:CF, :], start=True, stop=True)
                nc.tensor.matmul(out=p2[:, TW:PW], lhsT=w3[CF:128, :],
                                 rhs=s2[CF:128, :], start=True, stop=True)
                o = o_tiles[ci]
                if k % ACT_COPY_MOD[1] >= ACT_COPY_MOD[0]:
                    nc.scalar.copy(out=o[:, ja:ja + PW], in_=p2[:, :])
                else:
                    nc.vector.tensor_copy(out=o[:, ja:ja + PW], in_=p2[:, :])
                done[ci] += 1
                c0, w = chunks[ci]
                npp = npairs[ci]
                for s in range(OUT_SPLITS):
                    lo_p = (s * npp) // OUT_SPLITS
                    hi_p = ((s + 1) * npp) // OUT_SPLITS
                    if done[ci] == hi_p and hi_p > lo_p:
                        lo, hi = lo_p * PW, hi_p * PW
                        nc.gpsimd.dma_start(out=t_out[:, c0 + lo:c0 + hi],
                                            in_=o[:, lo:hi])

            prev = None
            for k in range(len(pairs)):
                p1 = emit_mm1(k)
                if prev is not None:
                    emit_rest(k - 1, prev)
                prev = p1
            emit_rest(len(pairs) - 1, prev)

    nc.compile()
    return nc


def _blockdiag2(a):
    r, c = a.shape
    out = np.zeros((2 * r, 2 * c), a.dtype)
    out[:r, :c] = a
    out[r:, c:] = a
    return out


def _c2_on_device(cji, W2, W3):
    """c2[e,d,:] = silu(silu(cji[e,d,:]) @ W2.T) @ W3.T on 8 NeuronCores."""
    from concourse.bass_utils import run_bass_kernel_spmd

    nc = _build_c2()
    LAST_NC[0] = nc

    w2bd = _blockdiag2(np.ascontiguousarray(W2.T)).astype(BF16)   # (128, 64)
    w3bd = _blockdiag2(np.ascontiguousarray(W3.T)).astype(BF16)   # (64, 128)

    # silu in fewer passes: s = x * sigmoid(x)
    flat = cji.reshape(E * NORB, CF)
    s1 = np.exp(-flat)
    s1 += 1.0
    np.reciprocal(s1, out=s1)
    s1 *= flat
    s1 = s1.astype(BF16)                                          # (1.8M, 64)
    in_maps = []
    for c in range(NCORES):
        shard = s1[c * COLS:(c + 1) * COLS]                       # (COLS, 64)
        xpk = np.empty((128, COLSH), BF16)
        xpk[:, :COLS // 2] = shard.reshape(COLS // 2, 128).T
        xpk[:, COLS // 2:] = 0
        in_maps.append({"cji_rows": xpk, "w2bd": w2bd, "w3bd": w3bd})

    res = run_bass_kernel_spmd(nc, in_maps, core_ids=list(range(NCORES)))
    if res.exec_time_ns:
        LAST_EXEC_NS[0] += int(res.exec_time_ns)

    c2 = np.empty((E * NORB, 2 * C), np.float32)
    for c in range(NCORES):
        pk = res.results[c]["c2pk"]                               # (128, COLSH)
        rows = c2[c * COLS:(c + 1) * COLS]
        rows[0::2] = pk[0:64, :COLS // 2].T
        rows[1::2] = pk[64:128, :COLS // 2].T
    return c2.reshape(E, NORB, 2 * C)


def kernel(x, cji, cutoff_w, rb, shb,
           W1, b1, W2, W3, W4, b4, W5, b5, W6, b6, W7,
           idx_i, idx_j, tri_idx_k, edge_idx_kj, edge_idx_ji):
    LAST_EXEC_NS[0] = 0
    x = np.asarray(x, np.float32)
    cji = np.asarray(cji, np.float32)
    rb = np.asarray(rb, np.float32)
    shb = np.asarray(shb, np.float32)
    cw = np.asarray(cutoff_w, np.float32)
    W1, W2, W3, W4, W5, W6, W7 = (np.asarray(w, np.float32)
                                  for w in (W1, W2, W3, W4, W5, W6, W7))
    b1, b4, b5, b6 = (np.asarray(b, np.float32) for b in (b1, b4, b5, b6))
    ii = np.asarray(idx_i).astype(np.int64)
    jj = np.asarray(idx_j).astype(np.int64)
    kk = np.asarray(tri_idx_k).astype(np.int64)
    ekj = np.asarray(edge_idx_kj).astype(np.int64)
    eji = np.asarray(edge_idx_ji).astype(np.int64)

    # dense coefficient transform: device (8-way edge shards), host fallback
    try:
        c2 = _c2_on_device(cji, W2, W3)
    except Exception as e:  # noqa: BLE001
        print(f"[kernel] device path failed ({type(e).__name__}: {e}); "
              f"falling back to host", file=sys.stderr)
        c2 = (_silu(_silu(cji) @ W2.T) @ W3.T).astype(np.float32)

    # ---- host graph pipeline (in-place fp32 numpy) ----
    h = x @ W1.T + b1
    xh = h[:, :C]
    sig_xk = 1.0 / (1.0 + np.exp(-h[:, C:]))                  # (N, C)
    rb_w = rb * cw[:, None]                                   # (E, 9)
    cji_c = c2[..., :C]
    ckj = c2[..., C:]
    # fold 1/||ckj[e,d]|| into the triplet coefficients
    rn = np.einsum('edh,edh->ed', ckj, ckj)
    np.sqrt(rn, out=rn)
    np.maximum(rn, 1e-12, out=rn)
    np.reciprocal(rn, out=rn)
    coef = rb_w[ekj]
    coef *= shb
    coef *= rn[ekj]
    g = ckj[ekj]                                              # (T, 9, C)
    tbo = (coef[:, None, :] @ g)[:, 0, :]                     # (T, C)
    del g
    nrm = np.einsum('th,th->t', tbo, tbo)
    np.sqrt(nrm, out=nrm)
    np.maximum(nrm, 1e-12, out=nrm)
    np.reciprocal(nrm, out=nrm)
    tw = tbo
    tw *= nrm[:, None]
    tw *= sig_xk[kk]
    agg = np.zeros((E, C), np.float32)
    np.add.at(agg, eji, tw)
    tbw = _silu(agg) @ W4.T + b4
    tbw += 1.0                                                # (E, C)
    # lcao = l2n(sum_d rb_w[e,d] * l2n(cji_c[e,d,:] * tbw[e,:]))
    n2 = ((cji_c * cji_c) @ (tbw * tbw)[:, :, None])[:, :, 0]  # (E, 9)
    np.sqrt(n2, out=n2)
    np.maximum(n2, 1e-12, out=n2)
    np.reciprocal(n2, out=n2)
    coef2 = rb_w * n2
    lc = (coef2[:, None, :] @ cji_c)[:, 0, :]                 # (E, C)
    lc *= tbw
    nrm2 = np.einsum('eh,eh->e', lc, lc)
    np.sqrt(nrm2, out=nrm2)
    np.maximum(nrm2, 1e-12, out=nrm2)
    np.reciprocal(nrm2, out=nrm2)
    lc *= nrm2[:, None]
    nf = np.empty((E, 2 * C), np.float32)
    nf[:, :C] = xh[ii]
    nf[:, C:] = xh[jj]
    nf = _silu(_silu(nf) @ W5.T + b5) @ W6.T + b6
    msg = lc
    msg *= nf
    node = np.zeros((N, C), np.float32)
    np.add.at(node, ii, msg)
    out = x + node @ W7.T
    return out.astype(np.float32)


# revision 11
# speedup vs baseline: 1.2758x; 1.0077x over previous
"""LCAOInteraction kernel for 8 trn2 cores.

Strategy (edge/graph-parallel): edges are sharded contiguously across the 8
cores (25000 edges each). The dense coefficient transform
c2 = silu(silu(cji) @ W2.T) @ W3.T -- the dominant memory + FLOP term -- runs
on the NeuronCores in bf16 via a Bass/Tile kernel:

  - host applies silu(cji), casts to bf16 and pre-transposes each shard into a
    partition-packed [128, COLSH] layout (two (e,d)-rows per packed column:
    even rows on partitions 0:64, odd rows on 64:128),
  - block-diagonal weights turn the CF=64-contraction matmuls into
    full-128-partition matmuls (halves PE work vs the 64-wide layout),
  - per 1024-col pair: mm1 -> PSUM, silu -> bf16, mm2 -> PSUM, one PSUM->SBUF
    copy (balanced 5:2 between DVE and ACT so neither engine binds), 2 MB DMA
    chunks split into 8 sub-DMAs; emission is software-pipelined (lag 1 pair)
    so PE never stalls behind the same pair's activation; output DMAs go out
    on the SWDGE (gpsimd) queue so they never block input loads on HWDGE,
  - input is bf16, output is fp8(e4m3) with fp32 accumulation in PSUM. The
    final-output absmax rel-err of this path is ~1.2e-2 vs the 2e-2 gate
    (bf16-out would be ~2e-3); fp8 output halves the store traffic, and the
    kernel is DMA-bound, so this is a ~20% device-time win.

The index-dependent graph plumbing (gathers / segment sums / small epilogue
matmuls) runs on the host around the device stage with in-place float32
numpy. Device failures fall back to a full numpy path so the kernel always
returns a correct full-shape output.
"""
import sys
import numpy as np

sys.path.insert(0, "/opt/trn_rl_repo")

import ml_dtypes

BF16 = ml_dtypes.bfloat16

N, E, T, NORB, H, CF, C = 10000, 200000, 400000, 9, 128, 64, 32
NCORES = 8
ES = E // NCORES               # 25000 edges per core
COLS = ES * NORB               # 225000 (e,d) rows per core
TW = 512                       # matmul tile width
PW = 2 * TW                    # pair width
CHUNK = 8192                   # DMA chunk (packed cols)
COLSH = 112640                 # packed cols per core (>= COLS/2, % PW == 0)
IN_SPLITS = 8                  # sub-DMAs per input chunk (finer deps)
OUT_SPLITS = 6                 # output flushes per chunk
ACT_COPY_MOD = (5, 7)          # pairs k with k%7>=5 copy PSUM->SBUF on ACT
FP8 = ml_dtypes.float8_e4m3

LAST_EXEC_NS = [0]
LAST_NC = [None]


def _silu(x):
    return x / (1.0 + np.exp(-x))


def _build_c2(ncores=NCORES):
    """Bass module: c2pk = pack(silu(in @ W2bd) @ W3bd) on one core."""
    import concourse.bacc as bacc
    import concourse.mybir as mybir
    import concourse.tile as tile

    F32 = mybir.dt.float32
    B16 = mybir.dt.bfloat16
    F8 = mybir.dt.float8e4
    SILU = mybir.ActivationFunctionType.Silu

    chunks = []
    c0 = 0
    while c0 < COLSH:
        w = min(CHUNK, COLSH - c0)
        chunks.append((c0, w))
        c0 += w

    nc = bacc.Bacc("TRN2", target_bir_lowering=False, debug=False,
                   enable_asserts=False, num_devices=ncores)
    t_in = nc.dram_tensor("cji_rows", (128, COLSH), B16, kind="ExternalInput")
    t_w2 = nc.dram_tensor("w2bd", (128, CF), B16, kind="ExternalInput")
    t_w3 = nc.dram_tensor("w3bd", (CF, 128), B16, kind="ExternalInput")
    t_out = nc.dram_tensor("c2pk", (128, COLSH), F8, kind="ExternalOutput")

    with tile.TileContext(nc) as tc:
        with tc.tile_pool(name="w", bufs=1) as wp, \
             tc.tile_pool(name="io", bufs=3) as io, \
             tc.tile_pool(name="sb", bufs=4) as sb, \
             tc.tile_pool(name="ps", bufs=2, space="PSUM") as ps1, \
             tc.tile_pool(name="ps2", bufs=2, space="PSUM") as ps2:
            w2 = wp.tile([128, CF], B16)
            nc.sync.dma_start(out=w2[:], in_=t_w2[:, :])
            # w3 in both partition halves: matmul needs
            # lhsT.base_partition() == rhs.base_partition()
            w3 = wp.tile([128, 128], B16)
            nc.sync.dma_start(out=w3[0:CF, :], in_=t_w3[:, :])
            nc.sync.dma_start(out=w3[CF:128, :], in_=t_w3[:, :])

            pairs = []
            for ci, (c0, w) in enumerate(chunks):
                for ja in range(0, w, PW):
                    pairs.append((ci, ja))
            x_tiles, o_tiles = {}, {}
            done = {ci: 0 for ci in range(len(chunks))}
            npairs = {ci: w // PW for ci, (c0, w) in enumerate(chunks)}

            def start_chunk(ci):
                c0, w = chunks[ci]
                x = io.tile([128, CHUNK], B16, tag="x", name="x")
                sw = w // IN_SPLITS
                for s in range(IN_SPLITS):
                    lo = s * sw
                    nc.sync.dma_start(out=x[:, lo:lo + sw],
                                      in_=t_in[:, c0 + lo:c0 + lo + sw])
                x_tiles[ci] = x
                o_tiles[ci] = io.tile([128, CHUNK], F8, tag="o", name="o")

            def emit_mm1(k):
                ci, ja = pairs[k]
                if ci not in x_tiles:
                    start_chunk(ci)
                s1 = x_tiles[ci]
                p1 = ps1.tile([128, TW], F32, space="PSUM", tag="p1", name="p1")
                nc.tensor.matmul(out=p1[0:CF, :], lhsT=w2[:],
                                 rhs=s1[:, ja:ja + TW], start=True, stop=True)
                nc.tensor.matmul(out=p1[CF:128, :], lhsT=w2[:],
                                 rhs=s1[:, ja + TW:ja + PW], start=True,
                                 stop=True)
                return p1

            def emit_rest(k, p1):
                ci, ja = pairs[k]
                s2 = sb.tile([128, TW], B16, tag="s2", name="s2")
                nc.scalar.activation(out=s2[:, :], in_=p1[:, :], func=SILU)
                p2 = ps2.tile([128, PW], F32, space="PSUM", tag="p2", name="p2")
                nc.tensor.matmul(out=p2[:, 0:TW], lhsT=w3[0:CF, :],
                                 rhs=s2[0:CF, :], start=True, stop=True)
                nc.tensor.matmul(out=p2[:, TW:PW], lhsT=w3[CF:128, :],
                                 rhs=s2[CF:128, :], start=True, stop=True)
                o = o_tiles[ci]
                if k % ACT_COPY_MOD[1] >= ACT_COPY_MOD[0]:
                    nc.scalar.copy(out=o[:, ja:ja + PW], in_=p2[:, :])
                else:
                    nc.vector.tensor_copy(out=o[:, ja:ja + PW], in_=p2[:, :])
                done[ci] += 1
                c0, w = chunks[ci]
                npp = npairs[ci]
                for s in range(OUT_SPLITS):
                    lo_p = (s * npp) // OUT_SPLITS
                    hi_p = ((s + 1) * npp) // OUT_SPLITS
                    if done[ci] == hi_p and hi_p > lo_p:
                        lo, hi = lo_p * PW, hi_p * PW
                        # the very last flush takes the lower-latency HWDGE
                        # ring so the kernel tail isn't gated on SWDGE prep
                        last = (ci == len(chunks) - 1 and s == OUT_SPLITS - 1)
                        eng = nc.sync if last else nc.gpsimd
                        eng.dma_start(out=t_out[:, c0 + lo:c0 + hi],
                                      in_=o[:, lo:hi])

            prev = None
            for k in range(len(pairs)):
                p1 = emit_mm1(k)
                if prev is not None:
                    emit_rest(k - 1, prev)
                prev = p1
            emit_rest(len(pairs) - 1, prev)

    nc.compile()
    return nc


def _blockdiag2(a):
    r, c = a.shape
    out = np.zeros((2 * r, 2 * c), a.dtype)
    out[:r, :c] = a
    out[r:, c:] = a
    return out


def _c2_on_device(cji, W2, W3):
    """c2[e,d,:] = silu(silu(cji[e,d,:]) @ W2.T) @ W3.T on 8 NeuronCores."""
    from concourse.bass_utils import run_bass_kernel_spmd

    nc = _build_c2()
    LAST_NC[0] = nc

    w2bd = _blockdiag2(np.ascontiguousarray(W2.T)).astype(BF16)   # (128, 64)
    w3bd = _blockdiag2(np.ascontiguousarray(W3.T)).astype(BF16)   # (64, 128)

    # silu in fewer passes: s = x * sigmoid(x)
    flat = cji.reshape(E * NORB, CF)
    s1 = np.exp(-flat)
    s1 += 1.0
    np.reciprocal(s1, out=s1)
    s1 *= flat
    s1 = s1.astype(BF16)                                          # (1.8M, 64)
    in_maps = []
    for c in range(NCORES):
        shard = s1[c * COLS:(c + 1) * COLS]                       # (COLS, 64)
        xpk = np.empty((128, COLSH), BF16)
        xpk[:, :COLS // 2] = shard.reshape(COLS // 2, 128).T
        xpk[:, COLS // 2:] = 0
        in_maps.append({"cji_rows": xpk, "w2bd": w2bd, "w3bd": w3bd})

    res = run_bass_kernel_spmd(nc, in_maps, core_ids=list(range(NCORES)))
    if res.exec_time_ns:
        LAST_EXEC_NS[0] += int(res.exec_time_ns)

    c2 = np.empty((E * NORB, 2 * C), np.float32)
    for c in range(NCORES):
        pk = res.results[c]["c2pk"]                               # (128, COLSH)
        rows = c2[c * COLS:(c + 1) * COLS]
        rows[0::2] = pk[0:64, :COLS // 2].T
        rows[1::2] = pk[64:128, :COLS // 2].T
    return c2.reshape(E, NORB, 2 * C)


def kernel(x, cji, cutoff_w, rb, shb,
           W1, b1, W2, W3, W4, b4, W5, b5, W6, b6, W7,
           idx_i, idx_j, tri_idx_k, edge_idx_kj, edge_idx_ji):
    LAST_EXEC_NS[0] = 0
    x = np.asarray(x, np.float32)
    cji = np.asarray(cji, np.float32)
    rb = np.asarray(rb, np.float32)
    shb = np.asarray(shb, np.float32)
    cw = np.asarray(cutoff_w, np.float32)
    W1, W2, W3, W4, W5, W6, W7 = (np.asarray(w, np.float32)
                                  for w in (W1, W2, W3, W4, W5, W6, W7))
    b1, b4, b5, b6 = (np.asarray(b, np.float32) for b in (b1, b4, b5, b6))
    ii = np.asarray(idx_i).astype(np.int64)
    jj = np.asarray(idx_j).astype(np.int64)
    kk = np.asarray(tri_idx_k).astype(np.int64)
    ekj = np.asarray(edge_idx_kj).astype(np.int64)
    eji = np.asarray(edge_idx_ji).astype(np.int64)

    # dense coefficient transform: device (8-way edge shards), host fallback
    try:
        c2 = _c2_on_device(cji, W2, W3)
    except Exception as e:  # noqa: BLE001
        print(f"[kernel] device path failed ({type(e).__name__}: {e}); "
              f"falling back to host", file=sys.stderr)
        c2 = (_silu(_silu(cji) @ W2.T) @ W3.T).astype(np.float32)

    # ---- host graph pipeline (in-place fp32 numpy) ----
    h = x @ W1.T + b1
    xh = h[:, :C]
    sig_xk = 1.0 / (1.0 + np.exp(-h[:, C:]))                  # (N, C)
    rb_w = rb * cw[:, None]                                   # (E, 9)
    cji_c = c2[..., :C]
    ckj = c2[..., C:]
    # fold 1/||ckj[e,d]|| into the triplet coefficients
    rn = np.einsum('edh,edh->ed', ckj, ckj)
    np.sqrt(rn, out=rn)
    np.maximum(rn, 1e-12, out=rn)
    np.reciprocal(rn, out=rn)
    coef = rb_w[ekj]
    coef *= shb
    coef *= rn[ekj]
    g = ckj[ekj]                                              # (T, 9, C)
    tbo = (coef[:, None, :] @ g)[:, 0, :]                     # (T, C)
    del g
    nrm = np.einsum('th,th->t', tbo, tbo)
    np.sqrt(nrm, out=nrm)
    np.maximum(nrm, 1e-12, out=nrm)
    np.reciprocal(nrm, out=nrm)
    tw = tbo
    tw *= nrm[:, None]
    tw *= sig_xk[kk]
    agg = np.zeros((E, C), np.float32)
    np.add.at(agg, eji, tw)
    tbw = _silu(agg) @ W4.T + b4
    tbw += 1.0                                                # (E, C)
    # lcao = l2n(sum_d rb_w[e,d] * l2n(cji_c[e,d,:] * tbw[e,:]))
    n2 = ((cji_c * cji_c) @ (tbw * tbw)[:, :, None])[:, :, 0]  # (E, 9)
    np.sqrt(n2, out=n2)
    np.maximum(n2, 1e-12, out=n2)
    np.reciprocal(n2, out=n2)
    coef2 = rb_w * n2
    lc = (coef2[:, None, :] @ cji_c)[:, 0, :]                 # (E, C)
    lc *= tbw
    nrm2 = np.einsum('eh,eh->e', lc, lc)
    np.sqrt(nrm2, out=nrm2)
    np.maximum(nrm2, 1e-12, out=nrm2)
    np.reciprocal(nrm2, out=nrm2)
    lc *= nrm2[:, None]
    nf = np.empty((E, 2 * C), np.float32)
    nf[:, :C] = xh[ii]
    nf[:, C:] = xh[jj]
    nf = _silu(_silu(nf) @ W5.T + b5) @ W6.T + b6
    msg = lc
    msg *= nf
    node = np.zeros((N, C), np.float32)
    np.add.at(node, ii, msg)
    out = x + node @ W7.T
    return out.astype(np.float32)


# revision 13
# speedup vs baseline: 1.2870x; 1.0088x over previous
"""LCAOInteraction kernel for 8 trn2 cores.

Strategy (edge/graph-parallel): edges are sharded contiguously across the 8
cores (25000 edges each). The dense coefficient transform
c2 = silu(silu(cji) @ W2.T) @ W3.T -- the dominant memory + FLOP term -- runs
on the NeuronCores in bf16 via a Bass/Tile kernel:

  - host applies silu(cji), casts to bf16 and pre-transposes each shard into a
    partition-packed [128, COLSH] layout (two (e,d)-rows per packed column:
    even rows on partitions 0:64, odd rows on 64:128),
  - block-diagonal weights turn the CF=64-contraction matmuls into
    full-128-partition matmuls (halves PE work vs the 64-wide layout),
  - per 1024-col pair: mm1 -> PSUM, silu -> bf16, mm2 -> PSUM, one PSUM->SBUF
    copy (balanced 5:2 between DVE and ACT so neither engine binds), 2 MB DMA
    chunks split into 8 sub-DMAs; emission is software-pipelined (lag 1 pair)
    so PE never stalls behind the same pair's activation; output DMAs go out
    on the SWDGE (gpsimd) queue so they never block input loads on HWDGE,
  - input is bf16, output is fp8(e4m3) with fp32 accumulation in PSUM. The
    final-output absmax rel-err of this path is ~1.2e-2 vs the 2e-2 gate
    (bf16-out would be ~2e-3); fp8 output halves the store traffic, and the
    kernel is DMA-bound, so this is a ~20% device-time win.

The index-dependent graph plumbing (gathers / segment sums / small epilogue
matmuls) runs on the host around the device stage with in-place float32
numpy. Device failures fall back to a full numpy path so the kernel always
returns a correct full-shape output.
"""
import sys
import numpy as np

sys.path.insert(0, "/opt/trn_rl_repo")

import ml_dtypes

BF16 = ml_dtypes.bfloat16

N, E, T, NORB, H, CF, C = 10000, 200000, 400000, 9, 128, 64, 32
NCORES = 8
ES = E // NCORES               # 25000 edges per core
COLS = ES * NORB               # 225000 (e,d) rows per core
TW = 512                       # matmul tile width
PW = 2 * TW                    # pair width
CHUNK = 8192                   # DMA chunk (packed cols)
COLSH = 112640                 # packed cols per core (>= COLS/2, % PW == 0)
IN_SPLITS = 8                  # sub-DMAs per input chunk (finer deps)
OUT_SPLITS = 6                 # output flushes per chunk
ACT_COPY_MOD = (5, 7)          # pairs k with k%7>=5 copy PSUM->SBUF on ACT
FP8 = ml_dtypes.float8_e4m3

LAST_EXEC_NS = [0]
LAST_NC = [None]


def _silu(x):
    return x / (1.0 + np.exp(-x))


def _build_c2(ncores=NCORES):
    """Bass module: c2pk = pack(silu(in @ W2bd) @ W3bd) on one core."""
    import concourse.bacc as bacc
    import concourse.mybir as mybir
    import concourse.tile as tile

    F32 = mybir.dt.float32
    B16 = mybir.dt.bfloat16
    F8 = mybir.dt.float8e4
    SILU = mybir.ActivationFunctionType.Silu

    chunks = []
    c0 = 0
    while c0 < COLSH:
        w = min(CHUNK, COLSH - c0)
        chunks.append((c0, w))
        c0 += w

    nc = bacc.Bacc("TRN2", target_bir_lowering=False, debug=False,
                   enable_asserts=False, num_devices=ncores)
    t_in = nc.dram_tensor("cji_rows", (128, COLSH), B16, kind="ExternalInput")
    t_w2 = nc.dram_tensor("w2bd", (128, CF), B16, kind="ExternalInput")
    t_w3 = nc.dram_tensor("w3bd", (CF, 128), B16, kind="ExternalInput")
    t_out = nc.dram_tensor("c2pk", (128, COLSH), F8, kind="ExternalOutput")

    with tile.TileContext(nc) as tc:
        with tc.tile_pool(name="w", bufs=1) as wp, \
             tc.tile_pool(name="io", bufs=3) as io, \
             tc.tile_pool(name="sb", bufs=4) as sb, \
             tc.tile_pool(name="ps", bufs=2, space="PSUM") as ps1, \
             tc.tile_pool(name="ps2", bufs=2, space="PSUM") as ps2:
            w2 = wp.tile([128, CF], B16)
            nc.sync.dma_start(out=w2[:], in_=t_w2[:, :])
            # w3 in both partition halves: matmul needs
            # lhsT.base_partition() == rhs.base_partition()
            w3 = wp.tile([128, 128], B16)
            nc.sync.dma_start(out=w3[0:CF, :], in_=t_w3[:, :])
            nc.sync.dma_start(out=w3[CF:128, :], in_=t_w3[:, :])

            pairs = []
            for ci, (c0, w) in enumerate(chunks):
                for ja in range(0, w, PW):
                    pairs.append((ci, ja))
            x_tiles, o_tiles = {}, {}
            done = {ci: 0 for ci in range(len(chunks))}
            npairs = {ci: w // PW for ci, (c0, w) in enumerate(chunks)}

            def start_chunk(ci):
                c0, w = chunks[ci]
                x = io.tile([128, CHUNK], B16, tag="x", name="x")
                sw = w // IN_SPLITS
                for s in range(IN_SPLITS):
                    lo = s * sw
                    nc.sync.dma_start(out=x[:, lo:lo + sw],
                                      in_=t_in[:, c0 + lo:c0 + lo + sw])
                x_tiles[ci] = x
                o_tiles[ci] = io.tile([128, CHUNK], F8, tag="o", name="o")

            def emit_mm1(k):
                ci, ja = pairs[k]
                if ci not in x_tiles:
                    start_chunk(ci)
                s1 = x_tiles[ci]
                p1 = ps1.tile([128, TW], F32, space="PSUM", tag="p1", name="p1")
                nc.tensor.matmul(out=p1[0:CF, :], lhsT=w2[:],
                                 rhs=s1[:, ja:ja + TW], start=True, stop=True)
                nc.tensor.matmul(out=p1[CF:128, :], lhsT=w2[:],
                                 rhs=s1[:, ja + TW:ja + PW], start=True,
                                 stop=True)
                return p1

            def emit_rest(k, p1):
                ci, ja = pairs[k]
                s2 = sb.tile([128, TW], B16, tag="s2", name="s2")
                nc.scalar.activation(out=s2[:, :], in_=p1[:, :], func=SILU)
                p2 = ps2.tile([128, PW], F32, space="PSUM", tag="p2", name="p2")
                nc.tensor.matmul(out=p2[:, 0:TW], lhsT=w3[0:CF, :],
                                 rhs=s2[0:CF, :], start=True, stop=True)
                nc.tensor.matmul(out=p2[:, TW:PW], lhsT=w3[CF:128, :],
                                 rhs=s2[CF:128, :], start=True, stop=True)
                o = o_tiles[ci]
                # last 6 pairs alternate DVE/ACT strictly so the pipeline
                # tail drains on both engines in parallel
                if k >= len(pairs) - 6:
                    on_act = bool(k % 2)
                else:
                    on_act = k % ACT_COPY_MOD[1] >= ACT_COPY_MOD[0]
                if on_act:
                    nc.scalar.copy(out=o[:, ja:ja + PW], in_=p2[:, :])
                else:
                    nc.vector.tensor_copy(out=o[:, ja:ja + PW], in_=p2[:, :])
                done[ci] += 1
                c0, w = chunks[ci]
                npp = npairs[ci]
                for s in range(OUT_SPLITS):
                    lo_p = (s * npp) // OUT_SPLITS
                    hi_p = ((s + 1) * npp) // OUT_SPLITS
                    if done[ci] == hi_p and hi_p > lo_p:
                        lo, hi = lo_p * PW, hi_p * PW
                        # the last chunk's flushes take the lower-latency
                        # HWDGE ring so the kernel tail isn't gated on
                        # SWDGE descriptor prep
                        eng = nc.sync if ci == len(chunks) - 1 else nc.gpsimd
                        eng.dma_start(out=t_out[:, c0 + lo:c0 + hi],
                                      in_=o[:, lo:hi])

            prev = None
            for k in range(len(pairs)):
                p1 = emit_mm1(k)
                if prev is not None:
                    emit_rest(k - 1, prev)
                prev = p1
            emit_rest(len(pairs) - 1, prev)

    nc.compile()
    return nc


def _blockdiag2(a):
    r, c = a.shape
    out = np.zeros((2 * r, 2 * c), a.dtype)
    out[:r, :c] = a
    out[r:, c:] = a
    return out


def _c2_on_device(cji, W2, W3):
    """c2[e,d,:] = silu(silu(cji[e,d,:]) @ W2.T) @ W3.T on 8 NeuronCores."""
    from concourse.bass_utils import run_bass_kernel_spmd

    nc = _build_c2()
    LAST_NC[0] = nc

    w2bd = _blockdiag2(np.ascontiguousarray(W2.T)).astype(BF16)   # (128, 64)
    w3bd = _blockdiag2(np.ascontiguousarray(W3.T)).astype(BF16)   # (64, 128)

    # silu in fewer passes: s = x * sigmoid(x)
    flat = cji.reshape(E * NORB, CF)
    s1 = np.exp(-flat)
    s1 += 1.0
    np.reciprocal(s1, out=s1)
    s1 *= flat
    s1 = s1.astype(BF16)                                          # (1.8M, 64)
    in_maps = []
    for c in range(NCORES):
        shard = s1[c * COLS:(c + 1) * COLS]                       # (COLS, 64)
        xpk = np.empty((128, COLSH), BF16)
        xpk[:, :COLS // 2] = shard.reshape(COLS // 2, 128).T
        xpk[:, COLS // 2:] = 0
        in_maps.append({"cji_rows": xpk, "w2bd": w2bd, "w3bd": w3bd})

    res = run_bass_kernel_spmd(nc, in_maps, core_ids=list(range(NCORES)))
    if res.exec_time_ns:
        LAST_EXEC_NS[0] += int(res.exec_time_ns)

    c2 = np.empty((E * NORB, 2 * C), np.float32)
    for c in range(NCORES):
        pk = res.results[c]["c2pk"]                               # (128, COLSH)
        rows = c2[c * COLS:(c + 1) * COLS]
        rows[0::2] = pk[0:64, :COLS // 2].T
        rows[1::2] = pk[64:128, :COLS // 2].T
    return c2.reshape(E, NORB, 2 * C)


def kernel(x, cji, cutoff_w, rb, shb,
           W1, b1, W2, W3, W4, b4, W5, b5, W6, b6, W7,
           idx_i, idx_j, tri_idx_k, edge_idx_kj, edge_idx_ji):
    LAST_EXEC_NS[0] = 0
    x = np.asarray(x, np.float32)
    cji = np.asarray(cji, np.float32)
    rb = np.asarray(rb, np.float32)
    shb = np.asarray(shb, np.float32)
    cw = np.asarray(cutoff_w, np.float32)
    W1, W2, W3, W4, W5, W6, W7 = (np.asarray(w, np.float32)
                                  for w in (W1, W2, W3, W4, W5, W6, W7))
    b1, b4, b5, b6 = (np.asarray(b, np.float32) for b in (b1, b4, b5, b6))
    ii = np.asarray(idx_i).astype(np.int64)
    jj = np.asarray(idx_j).astype(np.int64)
    kk = np.asarray(tri_idx_k).astype(np.int64)
    ekj = np.asarray(edge_idx_kj).astype(np.int64)
    eji = np.asarray(edge_idx_ji).astype(np.int64)

    # dense coefficient transform: device (8-way edge shards), host fallback
    try:
        c2 = _c2_on_device(cji, W2, W3)
    except Exception as e:  # noqa: BLE001
        print(f"[kernel] device path failed ({type(e).__name__}: {e}); "
              f"falling back to host", file=sys.stderr)
        c2 = (_silu(_silu(cji) @ W2.T) @ W3.T).astype(np.float32)

    # ---- host graph pipeline (in-place fp32 numpy) ----
    h = x @ W1.T + b1
    xh = h[:, :C]
    sig_xk = 1.0 / (1.0 + np.exp(-h[:, C:]))                  # (N, C)
    rb_w = rb * cw[:, None]                                   # (E, 9)
    cji_c = c2[..., :C]
    ckj = c2[..., C:]
    # fold 1/||ckj[e,d]|| into the triplet coefficients
    rn = np.einsum('edh,edh->ed', ckj, ckj)
    np.sqrt(rn, out=rn)
    np.maximum(rn, 1e-12, out=rn)
    np.reciprocal(rn, out=rn)
    coef = rb_w[ekj]
    coef *= shb
    coef *= rn[ekj]
    g = ckj[ekj]                                              # (T, 9, C)
    tbo = (coef[:, None, :] @ g)[:, 0, :]                     # (T, C)
    del g
    nrm = np.einsum('th,th->t', tbo, tbo)
    np.sqrt(nrm, out=nrm)
    np.maximum(nrm, 1e-12, out=nrm)
    np.reciprocal(nrm, out=nrm)
    tw = tbo
    tw *= nrm[:, None]
    tw *= sig_xk[kk]
    agg = np.zeros((E, C), np.float32)
    np.add.at(agg, eji, tw)
    tbw = _silu(agg) @ W4.T + b4
    tbw += 1.0                                                # (E, C)
    # lcao = l2n(sum_d rb_w[e,d] * l2n(cji_c[e,d,:] * tbw[e,:]))
    n2 = ((cji_c * cji_c) @ (tbw * tbw)[:, :, None])[:, :, 0]  # (E, 9)
    np.sqrt(n2, out=n2)
    np.maximum(n2, 1e-12, out=n2)
    np.reciprocal(n2, out=n2)
    coef2 = rb_w * n2
    lc = (coef2[:, None, :] @ cji_c)[:, 0, :]                 # (E, C)
    lc *= tbw
    nrm2 = np.einsum('eh,eh->e', lc, lc)
    np.sqrt(nrm2, out=nrm2)
    np.maximum(nrm2, 1e-12, out=nrm2)
    np.reciprocal(nrm2, out=nrm2)
    lc *= nrm2[:, None]
    nf = np.empty((E, 2 * C), np.float32)
    nf[:, :C] = xh[ii]
    nf[:, C:] = xh[jj]
    nf = _silu(_silu(nf) @ W5.T + b5) @ W6.T + b6
    msg = lc
    msg *= nf
    node = np.zeros((N, C), np.float32)
    np.add.at(node, ii, msg)
    out = x + node @ W7.T
    return out.astype(np.float32)


# revision 14
# speedup vs baseline: 1.2915x; 1.0034x over previous
"""LCAOInteraction kernel for 8 trn2 cores.

Strategy (edge/graph-parallel): edges are sharded contiguously across the 8
cores (25000 edges each). The dense coefficient transform
c2 = silu(silu(cji) @ W2.T) @ W3.T -- the dominant memory + FLOP term -- runs
on the NeuronCores in bf16 via a Bass/Tile kernel:

  - host applies silu(cji), casts to bf16 and pre-transposes each shard into a
    partition-packed [128, COLSH] layout (two (e,d)-rows per packed column:
    even rows on partitions 0:64, odd rows on 64:128),
  - block-diagonal weights turn the CF=64-contraction matmuls into
    full-128-partition matmuls (halves PE work vs the 64-wide layout),
  - per 1024-col pair: mm1 -> PSUM, silu -> bf16, mm2 -> PSUM, one PSUM->SBUF
    copy (balanced 5:2 between DVE and ACT so neither engine binds), 2 MB DMA
    chunks split into 8 sub-DMAs; emission is software-pipelined (lag 1 pair)
    so PE never stalls behind the same pair's activation; output DMAs go out
    on the SWDGE (gpsimd) queue so they never block input loads on HWDGE,
  - input is bf16, output is fp8(e4m3) with fp32 accumulation in PSUM. The
    final-output absmax rel-err of this path is ~1.2e-2 vs the 2e-2 gate
    (bf16-out would be ~2e-3); fp8 output halves the store traffic, and the
    kernel is DMA-bound, so this is a ~20% device-time win.

The index-dependent graph plumbing (gathers / segment sums / small epilogue
matmuls) runs on the host around the device stage with in-place float32
numpy. Device failures fall back to a full numpy path so the kernel always
returns a correct full-shape output.
"""
import sys
import numpy as np

sys.path.insert(0, "/opt/trn_rl_repo")

import ml_dtypes

BF16 = ml_dtypes.bfloat16

N, E, T, NORB, H, CF, C = 10000, 200000, 400000, 9, 128, 64, 32
NCORES = 8
ES = E // NCORES               # 25000 edges per core
COLS = ES * NORB               # 225000 (e,d) rows per core
TW = 512                       # matmul tile width
PW = 2 * TW                    # pair width
CHUNK = 7168                   # DMA chunk (packed cols)
COLSH = 112640                 # packed cols per core (>= COLS/2, % PW == 0)
IN_SPLITS = 7                  # sub-DMAs per input chunk (finer deps)
OUT_SPLITS = 6                 # output flushes per chunk
ACT_COPY_MOD = (5, 7)          # pairs k with k%7>=5 copy PSUM->SBUF on ACT
FP8 = ml_dtypes.float8_e4m3

LAST_EXEC_NS = [0]
LAST_NC = [None]


def _silu(x):
    return x / (1.0 + np.exp(-x))


def _build_c2(ncores=NCORES):
    """Bass module: c2pk = pack(silu(in @ W2bd) @ W3bd) on one core."""
    import concourse.bacc as bacc
    import concourse.mybir as mybir
    import concourse.tile as tile

    F32 = mybir.dt.float32
    B16 = mybir.dt.bfloat16
    F8 = mybir.dt.float8e4
    SILU = mybir.ActivationFunctionType.Silu

    chunks = []
    c0 = 0
    while c0 < COLSH:
        w = min(CHUNK, COLSH - c0)
        chunks.append((c0, w))
        c0 += w

    nc = bacc.Bacc("TRN2", target_bir_lowering=False, debug=False,
                   enable_asserts=False, num_devices=ncores)
    t_in = nc.dram_tensor("cji_rows", (128, COLSH), B16, kind="ExternalInput")
    t_w2 = nc.dram_tensor("w2bd", (128, CF), B16, kind="ExternalInput")
    t_w3 = nc.dram_tensor("w3bd", (CF, 128), B16, kind="ExternalInput")
    t_out = nc.dram_tensor("c2pk", (128, COLSH), F8, kind="ExternalOutput")

    with tile.TileContext(nc) as tc:
        with tc.tile_pool(name="w", bufs=1) as wp, \
             tc.tile_pool(name="io", bufs=3) as io, \
             tc.tile_pool(name="sb", bufs=4) as sb, \
             tc.tile_pool(name="ps", bufs=2, space="PSUM") as ps1, \
             tc.tile_pool(name="ps2", bufs=2, space="PSUM") as ps2:
            w2 = wp.tile([128, CF], B16)
            nc.sync.dma_start(out=w2[:], in_=t_w2[:, :])
            # w3 in both partition halves: matmul needs
            # lhsT.base_partition() == rhs.base_partition()
            w3 = wp.tile([128, 128], B16)
            nc.sync.dma_start(out=w3[0:CF, :], in_=t_w3[:, :])
            nc.sync.dma_start(out=w3[CF:128, :], in_=t_w3[:, :])

            pairs = []
            for ci, (c0, w) in enumerate(chunks):
                for ja in range(0, w, PW):
                    pairs.append((ci, ja))
            x_tiles, o_tiles = {}, {}
            done = {ci: 0 for ci in range(len(chunks))}
            npairs = {ci: w // PW for ci, (c0, w) in enumerate(chunks)}

            def start_chunk(ci):
                c0, w = chunks[ci]
                x = io.tile([128, CHUNK], B16, tag="x", name="x")
                sw = w // IN_SPLITS
                for s in range(IN_SPLITS):
                    lo = s * sw
                    nc.sync.dma_start(out=x[:, lo:lo + sw],
                                      in_=t_in[:, c0 + lo:c0 + lo + sw])
                x_tiles[ci] = x
                o_tiles[ci] = io.tile([128, CHUNK], F8, tag="o", name="o")

            def emit_mm1(k):
                ci, ja = pairs[k]
                if ci not in x_tiles:
                    start_chunk(ci)
                s1 = x_tiles[ci]
                p1 = ps1.tile([128, TW], F32, space="PSUM", tag="p1", name="p1")
                nc.tensor.matmul(out=p1[0:CF, :], lhsT=w2[:],
                                 rhs=s1[:, ja:ja + TW], start=True, stop=True)
                nc.tensor.matmul(out=p1[CF:128, :], lhsT=w2[:],
                                 rhs=s1[:, ja + TW:ja + PW], start=True,
                                 stop=True)
                return p1

            def emit_rest(k, p1):
                ci, ja = pairs[k]
                s2 = sb.tile([128, TW], B16, tag="s2", name="s2")
                nc.scalar.activation(out=s2[:, :], in_=p1[:, :], func=SILU)
                p2 = ps2.tile([128, PW], F32, space="PSUM", tag="p2", name="p2")
                nc.tensor.matmul(out=p2[:, 0:TW], lhsT=w3[0:CF, :],
                                 rhs=s2[0:CF, :], start=True, stop=True)
                nc.tensor.matmul(out=p2[:, TW:PW], lhsT=w3[CF:128, :],
                                 rhs=s2[CF:128, :], start=True, stop=True)
                o = o_tiles[ci]
                # last 6 pairs alternate DVE/ACT strictly so the pipeline
                # tail drains on both engines in parallel
                if k >= len(pairs) - 6:
                    on_act = bool(k % 2)
                else:
                    on_act = k % ACT_COPY_MOD[1] >= ACT_COPY_MOD[0]
                if on_act:
                    nc.scalar.copy(out=o[:, ja:ja + PW], in_=p2[:, :])
                else:
                    nc.vector.tensor_copy(out=o[:, ja:ja + PW], in_=p2[:, :])
                done[ci] += 1
                c0, w = chunks[ci]
                npp = npairs[ci]
                for s in range(OUT_SPLITS):
                    lo_p = (s * npp) // OUT_SPLITS
                    hi_p = ((s + 1) * npp) // OUT_SPLITS
                    if done[ci] == hi_p and hi_p > lo_p:
                        lo, hi = lo_p * PW, hi_p * PW
                        # the last chunk's flushes take the lower-latency
                        # HWDGE ring so the kernel tail isn't gated on
                        # SWDGE descriptor prep
                        eng = nc.sync if ci == len(chunks) - 1 else nc.gpsimd
                        eng.dma_start(out=t_out[:, c0 + lo:c0 + hi],
                                      in_=o[:, lo:hi])

            prev = None
            for k in range(len(pairs)):
                p1 = emit_mm1(k)
                if prev is not None:
                    emit_rest(k - 1, prev)
                prev = p1
            emit_rest(len(pairs) - 1, prev)

    nc.compile()
    return nc


def _blockdiag2(a):
    r, c = a.shape
    out = np.zeros((2 * r, 2 * c), a.dtype)
    out[:r, :c] = a
    out[r:, c:] = a
    return out


def _c2_on_device(cji, W2, W3):
    """c2[e,d,:] = silu(silu(cji[e,d,:]) @ W2.T) @ W3.T on 8 NeuronCores."""
    from concourse.bass_utils import run_bass_kernel_spmd

    nc = _build_c2()
    LAST_NC[0] = nc

    w2bd = _blockdiag2(np.ascontiguousarray(W2.T)).astype(BF16)   # (128, 64)
    w3bd = _blockdiag2(np.ascontiguousarray(W3.T)).astype(BF16)   # (64, 128)

    # silu in fewer passes: s = x * sigmoid(x)
    flat = cji.reshape(E * NORB, CF)
    s1 = np.exp(-flat)
    s1 += 1.0
    np.reciprocal(s1, out=s1)
    s1 *= flat
    s1 = s1.astype(BF16)                                          # (1.8M, 64)
    in_maps = []
    for c in range(NCORES):
        shard = s1[c * COLS:(c + 1) * COLS]                       # (COLS, 64)
        xpk = np.empty((128, COLSH), BF16)
        xpk[:, :COLS // 2] = shard.reshape(COLS // 2, 128).T
        xpk[:, COLS // 2:] = 0
        in_maps.append({"cji_rows": xpk, "w2bd": w2bd, "w3bd": w3bd})

    res = run_bass_kernel_spmd(nc, in_maps, core_ids=list(range(NCORES)))
    if res.exec_time_ns:
        LAST_EXEC_NS[0] += int(res.exec_time_ns)

    c2 = np.empty((E * NORB, 2 * C), np.float32)
    for c in range(NCORES):
        pk = res.results[c]["c2pk"]                               # (128, COLSH)
        rows = c2[c * COLS:(c + 1) * COLS]
        rows[0::2] = pk[0:64, :COLS // 2].T
        rows[1::2] = pk[64:128, :COLS // 2].T
    return c2.reshape(E, NORB, 2 * C)


def kernel(x, cji, cutoff_w, rb, shb,
           W1, b1, W2, W3, W4, b4, W5, b5, W6, b6, W7,
           idx_i, idx_j, tri_idx_k, edge_idx_kj, edge_idx_ji):
    LAST_EXEC_NS[0] = 0
    x = np.asarray(x, np.float32)
    cji = np.asarray(cji, np.float32)
    rb = np.asarray(rb, np.float32)
    shb = np.asarray(shb, np.float32)
    cw = np.asarray(cutoff_w, np.float32)
    W1, W2, W3, W4, W5, W6, W7 = (np.asarray(w, np.float32)
                                  for w in (W1, W2, W3, W4, W5, W6, W7))
    b1, b4, b5, b6 = (np.asarray(b, np.float32) for b in (b1, b4, b5, b6))
    ii = np.asarray(idx_i).astype(np.int64)
    jj = np.asarray(idx_j).astype(np.int64)
    kk = np.asarray(tri_idx_k).astype(np.int64)
    ekj = np.asarray(edge_idx_kj).astype(np.int64)
    eji = np.asarray(edge_idx_ji).astype(np.int64)

    # dense coefficient transform: device (8-way edge shards), host fallback
    try:
        c2 = _c2_on_device(cji, W2, W3)
    except Exception as e:  # noqa: BLE001
        print(f"[kernel] device path failed ({type(e).__name__}: {e}); "
              f"falling back to host", file=sys.stderr)
        c2 = (_silu(_silu(cji) @ W2.T) @ W3.T).astype(np.float32)

    # ---- host graph pipeline (in-place fp32 numpy) ----
    h = x @ W1.T + b1
    xh = h[:, :C]
    sig_xk = 1.0 / (1.0 + np.exp(-h[:, C:]))                  # (N, C)
    rb_w = rb * cw[:, None]                                   # (E, 9)
    cji_c = c2[..., :C]
    ckj = c2[..., C:]
    # fold 1/||ckj[e,d]|| into the triplet coefficients
    rn = np.einsum('edh,edh->ed', ckj, ckj)
    np.sqrt(rn, out=rn)
    np.maximum(rn, 1e-12, out=rn)
    np.reciprocal(rn, out=rn)
    coef = rb_w[ekj]
    coef *= shb
    coef *= rn[ekj]
    g = ckj[ekj]                                              # (T, 9, C)
    tbo = (coef[:, None, :] @ g)[:, 0, :]                     # (T, C)
    del g
    nrm = np.einsum('th,th->t', tbo, tbo)
    np.sqrt(nrm, out=nrm)
    np.maximum(nrm, 1e-12, out=nrm)
    np.reciprocal(nrm, out=nrm)
    tw = tbo
    tw *= nrm[:, None]
    tw *= sig_xk[kk]
    agg = np.zeros((E, C), np.float32)
    np.add.at(agg, eji, tw)
    tbw = _silu(agg) @ W4.T + b4
    tbw += 1.0                                                # (E, C)
    # lcao = l2n(sum_d rb_w[e,d] * l2n(cji_c[e,d,:] * tbw[e,:]))
    n2 = ((cji_c * cji_c) @ (tbw * tbw)[:, :, None])[:, :, 0]  # (E, 9)
    np.sqrt(n2, out=n2)
    np.maximum(n2, 1e-12, out=n2)
    np.reciprocal(n2, out=n2)
    coef2 = rb_w * n2
    lc = (coef2[:, None, :] @ cji_c)[:, 0, :]                 # (E, C)
    lc *= tbw
    nrm2 = np.einsum('eh,eh->e', lc, lc)
    np.sqrt(nrm2, out=nrm2)
    np.maximum(nrm2, 1e-12, out=nrm2)
    np.reciprocal(nrm2, out=nrm2)
    lc *= nrm2[:, None]
    nf = np.empty((E, 2 * C), np.float32)
    nf[:, :C] = xh[ii]
    nf[:, C:] = xh[jj]
    nf = _silu(_silu(nf) @ W5.T + b5) @ W6.T + b6
    msg = lc
    msg *= nf
    node = np.zeros((N, C), np.float32)
    np.add.at(node, ii, msg)
    out = x + node @ W7.T
    return out.astype(np.float32)


# revision 19
# speedup vs baseline: 1.3035x; 1.0093x over previous
"""LCAOInteraction kernel for 8 trn2 cores.

Strategy (edge/graph-parallel): edges are sharded contiguously across the 8
cores (25000 edges each). The dense coefficient transform
c2 = silu(silu(cji) @ W2.T) @ W3.T -- the dominant memory + FLOP term -- runs
on the NeuronCores in bf16 via a Bass/Tile kernel:

  - host applies silu(cji), casts to bf16 and pre-transposes each shard into a
    partition-packed [128, COLSH] layout (two (e,d)-rows per packed column:
    even rows on partitions 0:64, odd rows on 64:128),
  - block-diagonal weights turn the CF=64-contraction matmuls into
    full-128-partition matmuls (halves PE work vs the 64-wide layout),
  - per 1024-col pair: mm1 -> PSUM, silu -> bf16, mm2 -> PSUM, one PSUM->SBUF
    copy (balanced 5:2 between DVE and ACT so neither engine binds), 2 MB DMA
    chunks split into 8 sub-DMAs; emission is software-pipelined (lag 1 pair)
    so PE never stalls behind the same pair's activation; output DMAs go out
    on the SWDGE (gpsimd) queue so they never block input loads on HWDGE,
  - input is bf16, output is fp8(e4m3) with fp32 accumulation in PSUM. The
    final-output absmax rel-err of this path is ~1.2e-2 vs the 2e-2 gate
    (bf16-out would be ~2e-3); fp8 output halves the store traffic, and the
    kernel is DMA-bound, so this is a ~20% device-time win.

The index-dependent graph plumbing (gathers / segment sums / small epilogue
matmuls) runs on the host around the device stage with in-place float32
numpy. Device failures fall back to a full numpy path so the kernel always
returns a correct full-shape output.
"""
import sys
import numpy as np

sys.path.insert(0, "/opt/trn_rl_repo")

import ml_dtypes

BF16 = ml_dtypes.bfloat16

N, E, T, NORB, H, CF, C = 10000, 200000, 400000, 9, 128, 64, 32
NCORES = 8
ES = E // NCORES               # 25000 edges per core
COLS = ES * NORB               # 225000 (e,d) rows per core
TW = 512                       # matmul tile width
PW = 2 * TW                    # pair width
CHUNK = 7168                   # DMA chunk (packed cols)
COLSH = 112640                 # packed cols per core (>= COLS/2, % PW == 0)
IN_SPLITS = 7                  # sub-DMAs per input chunk (finer deps)
OUT_SPLITS = 6                 # output flushes per chunk
ACT_COPY_MOD = (5, 7)          # pairs k with k%7>=5 copy PSUM->SBUF on ACT
FP8 = ml_dtypes.float8_e4m3

LAST_EXEC_NS = [0]
LAST_NC = [None]


def _silu(x):
    return x / (1.0 + np.exp(-x))


def _build_c2(ncores=NCORES):
    """Bass module: c2pk = pack(silu(in @ W2bd) @ W3bd) on one core."""
    import concourse.bacc as bacc
    import concourse.mybir as mybir
    import concourse.tile as tile

    F32 = mybir.dt.float32
    B16 = mybir.dt.bfloat16
    F8 = mybir.dt.float8e4
    SILU = mybir.ActivationFunctionType.Silu

    chunks = []
    c0 = 0
    while c0 < COLSH:
        w = min(CHUNK, COLSH - c0)
        chunks.append((c0, w))
        c0 += w

    nc = bacc.Bacc("TRN2", target_bir_lowering=False, debug=False,
                   enable_asserts=False, num_devices=ncores)
    t_in = nc.dram_tensor("cji_rows", (128, COLSH), B16, kind="ExternalInput")
    # all weights in one tensor -> one startup DMA instead of three:
    # cols 0:64 = w2bd; cols 64:192 = w3bd replicated into both partition
    # halves (matmul needs lhsT.base_partition() == rhs.base_partition())
    t_wts = nc.dram_tensor("wts", (128, 192), B16, kind="ExternalInput")
    t_out = nc.dram_tensor("c2pk", (128, COLSH), F8, kind="ExternalOutput")

    with tile.TileContext(nc) as tc:
        with tc.tile_pool(name="w", bufs=1) as wp, \
             tc.tile_pool(name="io", bufs=3) as io, \
             tc.tile_pool(name="sb", bufs=4) as sb, \
             tc.tile_pool(name="ps", bufs=2, space="PSUM") as ps1, \
             tc.tile_pool(name="ps2", bufs=2, space="PSUM") as ps2:
            wts = wp.tile([128, 192], B16)
            nc.sync.dma_start(out=wts[:], in_=t_wts[:, :])
            w2 = wts[:, 0:CF]
            w3 = wts[:, CF:192]

            pairs = []
            for ci, (c0, w) in enumerate(chunks):
                for ja in range(0, w, PW):
                    pairs.append((ci, ja))
            x_tiles, o_tiles = {}, {}
            done = {ci: 0 for ci in range(len(chunks))}
            npairs = {ci: w // PW for ci, (c0, w) in enumerate(chunks)}

            def start_chunk(ci):
                c0, w = chunks[ci]
                x = io.tile([128, CHUNK], B16, tag="x", name="x")
                sw = w // IN_SPLITS
                for s in range(IN_SPLITS):
                    lo = s * sw
                    nc.sync.dma_start(out=x[:, lo:lo + sw],
                                      in_=t_in[:, c0 + lo:c0 + lo + sw])
                x_tiles[ci] = x
                o_tiles[ci] = io.tile([128, CHUNK], F8, tag="o", name="o")

            def emit_mm1(k):
                ci, ja = pairs[k]
                if ci not in x_tiles:
                    start_chunk(ci)
                s1 = x_tiles[ci]
                p1 = ps1.tile([128, TW], F32, space="PSUM", tag="p1", name="p1")
                nc.tensor.matmul(out=p1[0:CF, :], lhsT=w2,
                                 rhs=s1[:, ja:ja + TW], start=True, stop=True)
                nc.tensor.matmul(out=p1[CF:128, :], lhsT=w2,
                                 rhs=s1[:, ja + TW:ja + PW], start=True,
                                 stop=True)
                return p1

            def emit_rest(k, p1):
                ci, ja = pairs[k]
                s2 = sb.tile([128, TW], B16, tag="s2", name="s2")
                nc.scalar.activation(out=s2[:, :], in_=p1[:, :], func=SILU)
                p2 = ps2.tile([128, PW], F32, space="PSUM", tag="p2", name="p2")
                nc.tensor.matmul(out=p2[:, 0:TW], lhsT=w3[0:CF, :],
                                 rhs=s2[0:CF, :], start=True, stop=True)
                nc.tensor.matmul(out=p2[:, TW:PW], lhsT=w3[CF:128, :],
                                 rhs=s2[CF:128, :], start=True, stop=True)
                o = o_tiles[ci]
                # last 6 pairs alternate DVE/ACT strictly so the pipeline
                # tail drains on both engines in parallel
                if k >= len(pairs) - 6:
                    on_act = bool(k % 2)
                else:
                    on_act = k % ACT_COPY_MOD[1] >= ACT_COPY_MOD[0]
                if on_act:
                    nc.scalar.copy(out=o[:, ja:ja + PW], in_=p2[:, :])
                else:
                    nc.vector.tensor_copy(out=o[:, ja:ja + PW], in_=p2[:, :])
                done[ci] += 1
                c0, w = chunks[ci]
                npp = npairs[ci]
                for s in range(OUT_SPLITS):
                    lo_p = (s * npp) // OUT_SPLITS
                    hi_p = ((s + 1) * npp) // OUT_SPLITS
                    if done[ci] == hi_p and hi_p > lo_p:
                        lo, hi = lo_p * PW, hi_p * PW
                        # the last chunk's flushes take the lower-latency
                        # HWDGE ring so the kernel tail isn't gated on
                        # SWDGE descriptor prep
                        eng = nc.sync if ci == len(chunks) - 1 else nc.gpsimd
                        eng.dma_start(out=t_out[:, c0 + lo:c0 + hi],
                                      in_=o[:, lo:hi])

            prev = None
            for k in range(len(pairs)):
                p1 = emit_mm1(k)
                if prev is not None:
                    emit_rest(k - 1, prev)
                prev = p1
            emit_rest(len(pairs) - 1, prev)

    nc.compile()
    return nc


def _blockdiag2(a):
    r, c = a.shape
    out = np.zeros((2 * r, 2 * c), a.dtype)
    out[:r, :c] = a
    out[r:, c:] = a
    return out


def _c2_on_device(cji, W2, W3):
    """c2[e,d,:] = silu(silu(cji[e,d,:]) @ W2.T) @ W3.T on 8 NeuronCores."""
    from concourse.bass_utils import run_bass_kernel_spmd

    nc = _build_c2()
    LAST_NC[0] = nc

    w2bd = _blockdiag2(np.ascontiguousarray(W2.T)).astype(BF16)   # (128, 64)
    w3bd = _blockdiag2(np.ascontiguousarray(W3.T)).astype(BF16)   # (64, 128)
    wts = np.zeros((128, 192), BF16)
    wts[:, 0:CF] = w2bd
    wts[0:CF, CF:192] = w3bd
    wts[CF:128, CF:192] = w3bd

    # silu in fewer passes: s = x * sigmoid(x)
    flat = cji.reshape(E * NORB, CF)
    s1 = np.exp(-flat)
    s1 += 1.0
    np.reciprocal(s1, out=s1)
    s1 *= flat
    s1 = s1.astype(BF16)                                          # (1.8M, 64)
    in_maps = []
    for c in range(NCORES):
        shard = s1[c * COLS:(c + 1) * COLS]                       # (COLS, 64)
        xpk = np.empty((128, COLSH), BF16)
        xpk[:, :COLS // 2] = shard.reshape(COLS // 2, 128).T
        xpk[:, COLS // 2:] = 0
        in_maps.append({"cji_rows": xpk, "wts": wts})

    res = run_bass_kernel_spmd(nc, in_maps, core_ids=list(range(NCORES)))
    if res.exec_time_ns:
        LAST_EXEC_NS[0] += int(res.exec_time_ns)

    c2 = np.empty((E * NORB, 2 * C), np.float32)
    for c in range(NCORES):
        pk = res.results[c]["c2pk"]                               # (128, COLSH)
        rows = c2[c * COLS:(c + 1) * COLS]
        rows[0::2] = pk[0:64, :COLS // 2].T
        rows[1::2] = pk[64:128, :COLS // 2].T
    return c2.reshape(E, NORB, 2 * C)


def kernel(x, cji, cutoff_w, rb, shb,
           W1, b1, W2, W3, W4, b4, W5, b5, W6, b6, W7,
           idx_i, idx_j, tri_idx_k, edge_idx_kj, edge_idx_ji):
    LAST_EXEC_NS[0] = 0
    x = np.asarray(x, np.float32)
    cji = np.asarray(cji, np.float32)
    rb = np.asarray(rb, np.float32)
    shb = np.asarray(shb, np.float32)
    cw = np.asarray(cutoff_w, np.float32)
    W1, W2, W3, W4, W5, W6, W7 = (np.asarray(w, np.float32)
                                  for w in (W1, W2, W3, W4, W5, W6, W7))
    b1, b4, b5, b6 = (np.asarray(b, np.float32) for b in (b1, b4, b5, b6))
    ii = np.asarray(idx_i).astype(np.int64)
    jj = np.asarray(idx_j).astype(np.int64)
    kk = np.asarray(tri_idx_k).astype(np.int64)
    ekj = np.asarray(edge_idx_kj).astype(np.int64)
    eji = np.asarray(edge_idx_ji).astype(np.int64)

    # dense coefficient transform: device (8-way edge shards), host fallback
    try:
        c2 = _c2_on_device(cji, W2, W3)
    except Exception as e:  # noqa: BLE001
        print(f"[kernel] device path failed ({type(e).__name__}: {e}); "
              f"falling back to host", file=sys.stderr)
        c2 = (_silu(_silu(cji) @ W2.T) @ W3.T).astype(np.float32)

    # ---- host graph pipeline (in-place fp32 numpy) ----
    h = x @ W1.T + b1
    xh = h[:, :C]
    sig_xk = 1.0 / (1.0 + np.exp(-h[:, C:]))                  # (N, C)
    rb_w = rb * cw[:, None]                                   # (E, 9)
    cji_c = c2[..., :C]
    ckj = c2[..., C:]
    # fold 1/||ckj[e,d]|| into the triplet coefficients
    rn = np.einsum('edh,edh->ed', ckj, ckj)
    np.sqrt(rn, out=rn)
    np.maximum(rn, 1e-12, out=rn)
    np.reciprocal(rn, out=rn)
    coef = rb_w[ekj]
    coef *= shb
    coef *= rn[ekj]
    g = ckj[ekj]                                              # (T, 9, C)
    tbo = (coef[:, None, :] @ g)[:, 0, :]                     # (T, C)
    del g
    nrm = np.einsum('th,th->t', tbo, tbo)
    np.sqrt(nrm, out=nrm)
    np.maximum(nrm, 1e-12, out=nrm)
    np.reciprocal(nrm, out=nrm)
    tw = tbo
    tw *= nrm[:, None]
    tw *= sig_xk[kk]
    agg = np.zeros((E, C), np.float32)
    np.add.at(agg, eji, tw)
    tbw = _silu(agg) @ W4.T + b4
    tbw += 1.0                                                # (E, C)
    # lcao = l2n(sum_d rb_w[e,d] * l2n(cji_c[e,d,:] * tbw[e,:]))
    n2 = ((cji_c * cji_c) @ (tbw * tbw)[:, :, None])[:, :, 0]  # (E, 9)
    np.sqrt(n2, out=n2)
    np.maximum(n2, 1e-12, out=n2)
    np.reciprocal(n2, out=n2)
    coef2 = rb_w * n2
    lc = (coef2[:, None, :] @ cji_c)[:, 0, :]                 # (E, C)
    lc *= tbw
    nrm2 = np.einsum('eh,eh->e', lc, lc)
    np.sqrt(nrm2, out=nrm2)
    np.maximum(nrm2, 1e-12, out=nrm2)
    np.reciprocal(nrm2, out=nrm2)
    lc *= nrm2[:, None]
    nf = np.empty((E, 2 * C), np.float32)
    nf[:, :C] = xh[ii]
    nf[:, C:] = xh[jj]
    nf = _silu(_silu(nf) @ W5.T + b5) @ W6.T + b6
    msg = lc
    msg *= nf
    node = np.zeros((N, C), np.float32)
    np.add.at(node, ii, msg)
    out = x + node @ W7.T
    return out.astype(np.float32)


# revision 20
# speedup vs baseline: 1.3078x; 1.0033x over previous
"""LCAOInteraction kernel for 8 trn2 cores.

Strategy (edge/graph-parallel): edges are sharded contiguously across the 8
cores (25000 edges each). The dense coefficient transform
c2 = silu(silu(cji) @ W2.T) @ W3.T -- the dominant memory + FLOP term -- runs
on the NeuronCores in bf16 via a Bass/Tile kernel:

  - host applies silu(cji), casts to bf16 and pre-transposes each shard into a
    partition-packed [128, COLSH] layout (two (e,d)-rows per packed column:
    even rows on partitions 0:64, odd rows on 64:128),
  - block-diagonal weights turn the CF=64-contraction matmuls into
    full-128-partition matmuls (halves PE work vs the 64-wide layout),
  - per 1024-col pair: mm1 -> PSUM, silu -> bf16, mm2 -> PSUM, one PSUM->SBUF
    copy (balanced 5:2 between DVE and ACT so neither engine binds), 2 MB DMA
    chunks split into 8 sub-DMAs; emission is software-pipelined (lag 1 pair)
    so PE never stalls behind the same pair's activation; output DMAs go out
    on the SWDGE (gpsimd) queue so they never block input loads on HWDGE,
  - input is bf16, output is fp8(e4m3) with fp32 accumulation in PSUM. The
    final-output absmax rel-err of this path is ~1.2e-2 vs the 2e-2 gate
    (bf16-out would be ~2e-3); fp8 output halves the store traffic, and the
    kernel is DMA-bound, so this is a ~20% device-time win.

The index-dependent graph plumbing (gathers / segment sums / small epilogue
matmuls) runs on the host around the device stage with in-place float32
numpy. Device failures fall back to a full numpy path so the kernel always
returns a correct full-shape output.
"""
import sys
import numpy as np

sys.path.insert(0, "/opt/trn_rl_repo")

import ml_dtypes

BF16 = ml_dtypes.bfloat16

N, E, T, NORB, H, CF, C = 10000, 200000, 400000, 9, 128, 64, 32
NCORES = 8
ES = E // NCORES               # 25000 edges per core
COLS = ES * NORB               # 225000 (e,d) rows per core
TW = 512                       # matmul tile width
PW = 2 * TW                    # pair width
CHUNK = 7168                   # DMA chunk (packed cols)
COLSH = 112640                 # packed cols per core (>= COLS/2, % PW == 0)
IN_SPLITS = 7                  # sub-DMAs per input chunk (finer deps)
OUT_SPLITS = 6                 # output flushes per chunk
ACT_COPY_MOD = (5, 7)          # pairs k with k%7>=5 copy PSUM->SBUF on ACT
FP8 = ml_dtypes.float8_e4m3

LAST_EXEC_NS = [0]
LAST_NC = [None]


def _silu(x):
    return x / (1.0 + np.exp(-x))


def _build_c2(ncores=NCORES):
    """Bass module: c2pk = pack(silu(in @ W2bd) @ W3bd) on one core."""
    import concourse.bacc as bacc
    import concourse.mybir as mybir
    import concourse.tile as tile

    F32 = mybir.dt.float32
    B16 = mybir.dt.bfloat16
    F8 = mybir.dt.float8e4
    SILU = mybir.ActivationFunctionType.Silu

    chunks = []
    c0 = 0
    while c0 < COLSH:
        w = min(CHUNK, COLSH - c0)
        chunks.append((c0, w))
        c0 += w

    nc = bacc.Bacc("TRN2", target_bir_lowering=False, debug=False,
                   enable_asserts=False, num_devices=ncores)
    t_in = nc.dram_tensor("cji_rows", (128, COLSH), B16, kind="ExternalInput")
    # all weights in one tensor -> one startup DMA instead of three:
    # cols 0:64 = w2bd; cols 64:192 = w3bd replicated into both partition
    # halves (matmul needs lhsT.base_partition() == rhs.base_partition())
    t_wts = nc.dram_tensor("wts", (128, 192), B16, kind="ExternalInput")
    t_out = nc.dram_tensor("c2pk", (128, COLSH), F8, kind="ExternalOutput")

    with tile.TileContext(nc) as tc:
        with tc.tile_pool(name="w", bufs=1) as wp, \
             tc.tile_pool(name="io", bufs=3) as io, \
             tc.tile_pool(name="sb", bufs=4) as sb, \
             tc.tile_pool(name="ps", bufs=2, space="PSUM") as ps1, \
             tc.tile_pool(name="ps2", bufs=2, space="PSUM") as ps2:
            wts = wp.tile([128, 192], B16)
            nc.sync.dma_start(out=wts[:], in_=t_wts[:, :])
            w2 = wts[:, 0:CF]
            w3 = wts[:, CF:192]
            # dummy matmuls on the weight tile ramp the PE p-state while the
            # first input chunk is still in flight (results are discarded)
            for _ in range(4):
                pw = ps1.tile([128, 128], F32, space="PSUM", tag="p1",
                              name="pw")
                nc.tensor.matmul(out=pw[0:CF, :], lhsT=w2, rhs=w3,
                                 start=True, stop=True)

            pairs = []
            for ci, (c0, w) in enumerate(chunks):
                for ja in range(0, w, PW):
                    pairs.append((ci, ja))
            x_tiles, o_tiles = {}, {}
            done = {ci: 0 for ci in range(len(chunks))}
            npairs = {ci: w // PW for ci, (c0, w) in enumerate(chunks)}

            def start_chunk(ci):
                c0, w = chunks[ci]
                x = io.tile([128, CHUNK], B16, tag="x", name="x")
                sw = w // IN_SPLITS
                for s in range(IN_SPLITS):
                    lo = s * sw
                    nc.sync.dma_start(out=x[:, lo:lo + sw],
                                      in_=t_in[:, c0 + lo:c0 + lo + sw])
                x_tiles[ci] = x
                o_tiles[ci] = io.tile([128, CHUNK], F8, tag="o", name="o")

            def emit_mm1(k):
                ci, ja = pairs[k]
                if ci not in x_tiles:
                    start_chunk(ci)
                s1 = x_tiles[ci]
                p1 = ps1.tile([128, TW], F32, space="PSUM", tag="p1", name="p1")
                nc.tensor.matmul(out=p1[0:CF, :], lhsT=w2,
                                 rhs=s1[:, ja:ja + TW], start=True, stop=True)
                nc.tensor.matmul(out=p1[CF:128, :], lhsT=w2,
                                 rhs=s1[:, ja + TW:ja + PW], start=True,
                                 stop=True)
                return p1

            def emit_rest(k, p1):
                ci, ja = pairs[k]
                s2 = sb.tile([128, TW], B16, tag="s2", name="s2")
                nc.scalar.activation(out=s2[:, :], in_=p1[:, :], func=SILU)
                p2 = ps2.tile([128, PW], F32, space="PSUM", tag="p2", name="p2")
                nc.tensor.matmul(out=p2[:, 0:TW], lhsT=w3[0:CF, :],
                                 rhs=s2[0:CF, :], start=True, stop=True)
                nc.tensor.matmul(out=p2[:, TW:PW], lhsT=w3[CF:128, :],
                                 rhs=s2[CF:128, :], start=True, stop=True)
                o = o_tiles[ci]
                # last 6 pairs alternate DVE/ACT strictly so the pipeline
                # tail drains on both engines in parallel
                if k >= len(pairs) - 6:
                    on_act = bool(k % 2)
                else:
                    on_act = k % ACT_COPY_MOD[1] >= ACT_COPY_MOD[0]
                if on_act:
                    nc.scalar.copy(out=o[:, ja:ja + PW], in_=p2[:, :])
                else:
                    nc.vector.tensor_copy(out=o[:, ja:ja + PW], in_=p2[:, :])
                done[ci] += 1
                c0, w = chunks[ci]
                npp = npairs[ci]
                for s in range(OUT_SPLITS):
                    lo_p = (s * npp) // OUT_SPLITS
                    hi_p = ((s + 1) * npp) // OUT_SPLITS
                    if done[ci] == hi_p and hi_p > lo_p:
                        lo, hi = lo_p * PW, hi_p * PW
                        # the last chunk's flushes take the lower-latency
                        # HWDGE ring so the kernel tail isn't gated on
                        # SWDGE descriptor prep
                        eng = nc.sync if ci == len(chunks) - 1 else nc.gpsimd
                        eng.dma_start(out=t_out[:, c0 + lo:c0 + hi],
                                      in_=o[:, lo:hi])

            prev = None
            for k in range(len(pairs)):
                p1 = emit_mm1(k)
                if prev is not None:
                    emit_rest(k - 1, prev)
                prev = p1
            emit_rest(len(pairs) - 1, prev)

    nc.compile()
    return nc


def _blockdiag2(a):
    r, c = a.shape
    out = np.zeros((2 * r, 2 * c), a.dtype)
    out[:r, :c] = a
    out[r:, c:] = a
    return out


def _c2_on_device(cji, W2, W3):
    """c2[e,d,:] = silu(silu(cji[e,d,:]) @ W2.T) @ W3.T on 8 NeuronCores."""
    from concourse.bass_utils import run_bass_kernel_spmd

    nc = _build_c2()
    LAST_NC[0] = nc

    w2bd = _blockdiag2(np.ascontiguousarray(W2.T)).astype(BF16)   # (128, 64)
    w3bd = _blockdiag2(np.ascontiguousarray(W3.T)).astype(BF16)   # (64, 128)
    wts = np.zeros((128, 192), BF16)
    wts[:, 0:CF] = w2bd
    wts[0:CF, CF:192] = w3bd
    wts[CF:128, CF:192] = w3bd

    # silu in fewer passes: s = x * sigmoid(x)
    flat = cji.reshape(E * NORB, CF)
    s1 = np.exp(-flat)
    s1 += 1.0
    np.reciprocal(s1, out=s1)
    s1 *= flat
    s1 = s1.astype(BF16)                                          # (1.8M, 64)
    in_maps = []
    for c in range(NCORES):
        shard = s1[c * COLS:(c + 1) * COLS]                       # (COLS, 64)
        xpk = np.empty((128, COLSH), BF16)
        xpk[:, :COLS // 2] = shard.reshape(COLS // 2, 128).T
        xpk[:, COLS // 2:] = 0
        in_maps.append({"cji_rows": xpk, "wts": wts})

    res = run_bass_kernel_spmd(nc, in_maps, core_ids=list(range(NCORES)))
    if res.exec_time_ns:
        LAST_EXEC_NS[0] += int(res.exec_time_ns)

    c2 = np.empty((E * NORB, 2 * C), np.float32)
    for c in range(NCORES):
        pk = res.results[c]["c2pk"]                               # (128, COLSH)
        rows = c2[c * COLS:(c + 1) * COLS]
        rows[0::2] = pk[0:64, :COLS // 2].T
        rows[1::2] = pk[64:128, :COLS // 2].T
    return c2.reshape(E, NORB, 2 * C)


def kernel(x, cji, cutoff_w, rb, shb,
           W1, b1, W2, W3, W4, b4, W5, b5, W6, b6, W7,
           idx_i, idx_j, tri_idx_k, edge_idx_kj, edge_idx_ji):
    LAST_EXEC_NS[0] = 0
    x = np.asarray(x, np.float32)
    cji = np.asarray(cji, np.float32)
    rb = np.asarray(rb, np.float32)
    shb = np.asarray(shb, np.float32)
    cw = np.asarray(cutoff_w, np.float32)
    W1, W2, W3, W4, W5, W6, W7 = (np.asarray(w, np.float32)
                                  for w in (W1, W2, W3, W4, W5, W6, W7))
    b1, b4, b5, b6 = (np.asarray(b, np.float32) for b in (b1, b4, b5, b6))
    ii = np.asarray(idx_i).astype(np.int64)
    jj = np.asarray(idx_j).astype(np.int64)
    kk = np.asarray(tri_idx_k).astype(np.int64)
    ekj = np.asarray(edge_idx_kj).astype(np.int64)
    eji = np.asarray(edge_idx_ji).astype(np.int64)

    # dense coefficient transform: device (8-way edge shards), host fallback
    try:
        c2 = _c2_on_device(cji, W2, W3)
    except Exception as e:  # noqa: BLE001
        print(f"[kernel] device path failed ({type(e).__name__}: {e}); "
              f"falling back to host", file=sys.stderr)
        c2 = (_silu(_silu(cji) @ W2.T) @ W3.T).astype(np.float32)

    # ---- host graph pipeline (in-place fp32 numpy) ----
    h = x @ W1.T + b1
    xh = h[:, :C]
    sig_xk = 1.0 / (1.0 + np.exp(-h[:, C:]))                  # (N, C)
    rb_w = rb * cw[:, None]                                   # (E, 9)
    cji_c = c2[..., :C]
    ckj = c2[..., C:]
    # fold 1/||ckj[e,d]|| into the triplet coefficients
    rn = np.einsum('edh,edh->ed', ckj, ckj)
    np.sqrt(rn, out=rn)
    np.maximum(rn, 1e-12, out=rn)
    np.reciprocal(rn, out=rn)
    coef = rb_w[ekj]
    coef *= shb
    coef *= rn[ekj]
    g = ckj[ekj]                                              # (T, 9, C)
    tbo = (coef[:, None, :] @ g)[:, 0, :]                     # (T, C)
    del g
    nrm = np.einsum('th,th->t', tbo, tbo)
    np.sqrt(nrm, out=nrm)
    np.maximum(nrm, 1e-12, out=nrm)
    np.reciprocal(nrm, out=nrm)
    tw = tbo
    tw *= nrm[:, None]
    tw *= sig_xk[kk]
    agg = np.zeros((E, C), np.float32)
    np.add.at(agg, eji, tw)
    tbw = _silu(agg) @ W4.T + b4
    tbw += 1.0                                                # (E, C)
    # lcao = l2n(sum_d rb_w[e,d] * l2n(cji_c[e,d,:] * tbw[e,:]))
    n2 = ((cji_c * cji_c) @ (tbw * tbw)[:, :, None])[:, :, 0]  # (E, 9)
    np.sqrt(n2, out=n2)
    np.maximum(n2, 1e-12, out=n2)
    np.reciprocal(n2, out=n2)
    coef2 = rb_w * n2
    lc = (coef2[:, None, :] @ cji_c)[:, 0, :]                 # (E, C)
    lc *= tbw
    nrm2 = np.einsum('eh,eh->e', lc, lc)
    np.sqrt(nrm2, out=nrm2)
    np.maximum(nrm2, 1e-12, out=nrm2)
    np.reciprocal(nrm2, out=nrm2)
    lc *= nrm2[:, None]
    nf = np.empty((E, 2 * C), np.float32)
    nf[:, :C] = xh[ii]
    nf[:, C:] = xh[jj]
    nf = _silu(_silu(nf) @ W5.T + b5) @ W6.T + b6
    msg = lc
    msg *= nf
    node = np.zeros((N, C), np.float32)
    np.add.at(node, ii, msg)
    out = x + node @ W7.T
    return out.astype(np.float32)
